# revision 23
# baseline (speedup 1.0000x reference)
"""OGRENet GNN message-passing kernel for 8 Trainium2 NeuronCores.

Strategy (v2)
-------------
Host (numpy, cheap index plumbing only):
  * u2 = u @ Wsel + bsel (64x256). Graph-level folds: the u2 contribution to
    edge-MLP L1 is (u2 @ eW1_u)[g] over only 64 graphs -> a 64x1024 bias
    table selected on-device by a one-hot(graph) block inside the L1 K-tile.
    Same trick for node-MLP2's u2 part (64x512 table). Biases ride in a
    const-1 row of the same K-tile, so L1 needs ONE K=128 fp16 matmul per
    M-tile (was 3).
  * eW5 feeds only node-MLP1 -> fold W_comb = eW5 @ n1W1[9:521] on host and
    skip the e5 layer entirely (bias folded into n1b1').
  * sort edges by destination (`row`), split at node boundaries into 8
    contiguous ranges (2500 nodes/core) -> no cross-core reduction.

Device (per core, identical program, different data):
  * edge MLP L2..L4 as fp8 DoubleRow matmuls (weights x64, activations raw
    fp8). ACT does the biased relus into fp8 pairs; L1/n1a relus are split
    ACT/DVE so psum-bank drains keep pace with the PE.
  * node MLP1 L2 flips edge-major; segment mean becomes a matmul with a
    membership mask as the stationary operand (DVE is_equal builds the mask
    with the window shift folded into the op), 128-node windows; agg is
    flipped back feature-major with PE transposes for node-MLP2.
  * emission is software-pipelined with a 4-half L1 lookahead; ready window
    work is interleaved at the L3/L4 boundary and membership masks are
    front-loaded in the DVE queue so the PE instruction stream stays
    gapless (any idle also drops the PE p-state, doubling the cost).
"""

import os
import sys

import numpy as np

sys.path.insert(0, "/opt/trn_rl_repo")

N_NODES = 20000
N_GRAPHS = 64
U_DIM = 256
E_HID = 1024
N_HID = 512
NC = 8
NPN = N_NODES // NC          # nodes per core (2500)
NPAD = 2560                  # padded nodes per core
WN = 128                     # nodes per segment window
NWIN = NPAD // WN            # 20
P = 128

_CACHE = {}


def _pack_cols(v, T):
    """[T*128] -> [128, T] with col t = v[t*128:(t+1)*128]."""
    return np.ascontiguousarray(v.reshape(T, P).T)


def _build_module(EPAD, win_tiles, h2_bufs, memb_bufs):
    """Build the per-core Bass program. win_tiles[w] = (tlo, thi) inclusive."""
    from concourse import bacc, mybir, tile

    T = EPAD // P            # 128-edge tiles
    NB = EPAD // 1024        # 1024-edge dma blocks
    NH = EPAD // 512         # 512-edge halves
    f16 = mybir.dt.float16
    f32 = mybir.dt.float32
    f8 = mybir.dt.float8e4
    RELU = mybir.ActivationFunctionType.Relu
    COPY = mybir.ActivationFunctionType.Copy
    IDENT = mybir.ActivationFunctionType.Identity
    DR = mybir.MatmulPerfMode.DoubleRow
    ADD = mybir.AluOpType.add
    MAX = mybir.AluOpType.max
    MULT = mybir.AluOpType.mult
    ISEQ = mybir.AluOpType.is_equal
    INV64 = 1.0 / 64.0

    nc = bacc.Bacc(None, target_bir_lowering=False, debug=False)

    with tile.TileContext(nc) as tc:
        with (
            tc.tile_pool(name="dram", bufs=1, space="DRAM") as dram,
            tc.tile_pool(name="wres", bufs=1) as wres,
            tc.tile_pool(name="einp", bufs=4) as einp,
            tc.tile_pool(name="actp", bufs=16) as actp,
            tc.tile_pool(name="a1pool", bufs=20) as a1pool,
            tc.tile_pool(name="h2p", bufs=h2_bufs) as h2p,
            tc.tile_pool(name="membp", bufs=memb_bufs) as membp,
            tc.tile_pool(name="tmpp", bufs=3) as tmpp,
            tc.tile_pool(name="zp", bufs=6) as zp,
            tc.tile_pool(name="aggp", bufs=8) as aggp,
            tc.tile_pool(name="bigps", bufs=6, space="PSUM") as bigps,
            tc.tile_pool(name="segps", bufs=2, space="PSUM") as segps,
        ):
            # ---- DRAM I/O -------------------------------------------------
            d_ein = dram.tile([P, EPAD], f16, kind="ExternalInput", name="ein")
            d_relw = dram.tile([P, T], f32, kind="ExternalInput", name="relw")
            d_invc = dram.tile([P, T], f32, kind="ExternalInput", name="invc")
            d_xn = dram.tile([P, NPAD], f16, kind="ExternalInput", name="xn")
            d_iota = dram.tile([P, WN], f16, kind="ExternalInput", name="iota")
            d_ident = dram.tile([P, P], f16, kind="ExternalInput", name="ident")
            d_n1b2bc = dram.tile([P, 512], f32, kind="ExternalInput", name="n1b2bc")

            wspec16 = dict(W1c=[P, E_HID], W6c=[P, N_HID], W8c=[P, N_HID],
                           W8a=[N_HID, N_HID], W9=[N_HID, 1])
            d_w = {k: dram.tile(s, f16, kind="ExternalInput", name=k)
                   for k, s in wspec16.items()}
            for k in ("eW2", "eW3", "eW4"):
                d_w[k] = dram.tile([P, 8192], f8, kind="ExternalInput", name=k)
            d_w["Wcomb"] = dram.tile([P, 4096], f8, kind="ExternalInput", name="Wcomb")
            d_w["n1W2"] = dram.tile([P, 2048], f8, kind="ExternalInput", name="n1W2")
            bspec = dict(eb2r=[P, 8], eb3r=[P, 8], eb4r=[P, 8], n2b2r=[1, 1])
            d_b = {k: dram.tile(s, f32, kind="ExternalInput", name=k)
                   for k, s in bspec.items()}
            d_z = dram.tile([1, NPAD], f32, kind="ExternalOutput", name="zout")

            names = dict(ein=d_ein.name, relw=d_relw.name, invc=d_invc.name,
                         xn=d_xn.name, iota=d_iota.name, ident=d_ident.name,
                         n1b2bc=d_n1b2bc.name, zout=d_z.name)
            names.update({k: v.name for k, v in d_w.items()})
            names.update({k: v.name for k, v in d_b.items()})

            # ---- resident loads ------------------------------------------
            def load_w(name, nk, width):
                ts = []
                for k in range(nk):
                    t = wres.tile([P, width], f16, name=f"w_{name}_{k}")
                    nc.sync.dma_start(out=t[:], in_=d_w[name][k * P:(k + 1) * P, :])
                    ts.append(t)
                return ts

            def load_wdr(name, npair, width):
                ts = []
                for q in range(npair):
                    t = wres.tile([P, 2, width], f8, name=f"w_{name}_{q}")
                    nc.sync.dma_start(
                        out=t[:, :, :],
                        in_=d_w[name][:, q * 2 * width:(q + 1) * 2 * width])
                    ts.append(t)
                return ts

            # prefetch the first two edge-input blocks ahead of the bulkier
            # weight loads so L1(0) isn't DMA-bound at startup
            ein_blocks = {}
            for b0 in range(min(4, NB)):
                t = einp.tile([P, 1024], f16, name="ein_t")
                for hh in range(2):
                    nc.sync.dma_start(
                        out=t[:, hh * 512:(hh + 1) * 512],
                        in_=d_ein[:, b0 * 1024 + hh * 512:
                                  b0 * 1024 + (hh + 1) * 512])
                ein_blocks[b0] = t

            W1c = load_w("W1c", 1, E_HID)[0]
            W2 = load_wdr("eW2", 4, E_HID)
            W3 = load_wdr("eW3", 4, E_HID)
            W4 = load_wdr("eW4", 4, E_HID)
            Wcb = load_wdr("Wcomb", 4, N_HID)
            W6c = load_w("W6c", 1, N_HID)[0]
            W7 = load_wdr("n1W2", 2, N_HID)
            W8c = load_w("W8c", 1, N_HID)[0]
            W8a = load_w("W8a", 4, N_HID)
            W9 = load_w("W9", 4, 1)

            B = {}
            for k, s in bspec.items():
                t = wres.tile(s, f32, name=f"b_{k}")
                nc.sync.dma_start(out=t[:], in_=d_b[k][:])
                B[k] = t
            relw = wres.tile([P, T], f32, name="relw_sb")
            nc.sync.dma_start(out=relw[:], in_=d_relw[:])
            invc = wres.tile([P, T], f32, name="invc_sb")
            nc.sync.dma_start(out=invc[:], in_=d_invc[:])
            iota = wres.tile([P, WN], f16, name="iota_sb")
            nc.sync.dma_start(out=iota[:], in_=d_iota[:])
            ident = wres.tile([P, P], f16, name="ident_sb")
            nc.sync.dma_start(out=ident[:], in_=d_ident[:])
            n1b2bc = wres.tile([P, 512], f32, name="n1b2bc_sb")
            nc.sync.dma_start(out=n1b2bc[:], in_=d_n1b2bc[:])
            xn = wres.tile([P, NPAD], f16, name="xn_sb")
            nc.sync.dma_start(out=xn[:], in_=d_xn[:])

            # ---- state ----------------------------------------------------
            a1_pairs = {}      # h -> [4 pair tiles]
            h2_tiles = {}      # global tile idx -> tile

            def emit_L1(h):
                b = h // 2
                if h % 2 == 0 and b not in ein_blocks:
                    t = einp.tile([P, 1024], f16, name="ein_t")
                    nc.sync.dma_start(out=t[:],
                                      in_=d_ein[:, b * 1024:(b + 1) * 1024])
                    ein_blocks[b] = t
                einh = ein_blocks[b][:, (h % 2) * 512:(h % 2) * 512 + 512]
                pairs = []
                pt = None
                for m in range(8):
                    ps = bigps.tile([P, 512], mybir.dt.float32, name="ps_big")
                    nc.tensor.matmul(out=ps[:], lhsT=W1c[:, m * P:(m + 1) * P],
                                     rhs=einh, start=True, stop=True)
                    if m % 2 == 0:
                        pt = a1pool.tile([P, 2, 512], f8, name="pairA")
                        pairs.append(pt)
                    if m in (0, 1, 4, 5):
                        nc.scalar.activation(pt[:, m % 2, :], ps[:], RELU)
                    else:
                        nc.vector.tensor_scalar(
                            out=pt[:, m % 2, :], in0=ps[:], scalar1=0.0,
                            scalar2=None, op0=MAX)
                a1_pairs[h] = pairs
                return einh

            def dr_layer(pin, Wp, bias, tag):
                outs = []
                pt = None
                # interleave the first two M-groups so the group-0 q=3 matmul
                # lands after the previous layer's last activation drains
                ps0 = bigps.tile([P, 512], mybir.dt.float32, name="ps_big")
                ps1 = bigps.tile([P, 512], mybir.dt.float32, name="ps_big")
                for q in range(4):
                    for m, ps in ((0, ps0), (1, ps1)):
                        nc.tensor.matmul(
                            out=ps[:], lhsT=Wp[q][:, :, m * P:(m + 1) * P],
                            rhs=pin[q][:, :, :], start=(q == 0),
                            stop=(q == 3), perf_mode=DR)
                for m in range(8):
                    if m == 0:
                        ps = ps0
                    elif m == 1:
                        ps = ps1
                    else:
                        ps = bigps.tile([P, 512], mybir.dt.float32, name="ps_big")
                        for q in range(4):
                            nc.tensor.matmul(
                                out=ps[:], lhsT=Wp[q][:, :, m * P:(m + 1) * P],
                                rhs=pin[q][:, :, :], start=(q == 0),
                                stop=(q == 3), perf_mode=DR)
                    if m % 2 == 0:
                        pt = actp.tile([P, 2, 512], f8, name=tag)
                        outs.append(pt)
                    nc.scalar.activation(pt[:, m % 2, :], ps[:], RELU,
                                         bias=bias[:, m:m + 1], scale=INV64)
                return outs

            def emit_rest(h, einh, emit_filler):
                a2 = dr_layer(a1_pairs.pop(h), W2, B["eb2r"], "pairB")
                a3 = dr_layer(a2, W3, B["eb3r"], "pairC")
                emit_filler()
                a4 = dr_layer(a3, W4, B["eb4r"], "pairB")
                # node MLP1 L1: fp16 x/bias part + fp8 DR W_comb on a4
                h1p = []
                pt = None
                for m in range(4):
                    ps = bigps.tile([P, 512], mybir.dt.float32, name="ps_big")
                    nc.tensor.matmul(out=ps[:], lhsT=W6c[:, m * P:(m + 1) * P],
                                     rhs=einh, start=True, stop=False)
                    for q in range(4):
                        nc.tensor.matmul(
                            out=ps[:], lhsT=Wcb[q][:, :, m * P:(m + 1) * P],
                            rhs=a4[q][:, :, :], start=False, stop=(q == 3),
                            perf_mode=DR)
                    if m % 2 == 0:
                        pt = actp.tile([P, 2, 512], f8, name="pairD")
                        h1p.append(pt)
                    if m < 2:
                        nc.scalar.activation(pt[:, m % 2, :], ps[:], RELU,
                                             scale=INV64)
                    else:
                        nc.vector.tensor_scalar(
                            out=pt[:, m % 2, :], in0=ps[:], scalar1=INV64,
                            scalar2=0.0, op0=MULT, op1=MAX)
                # node MLP1 L2 -> edge-major h2 tiles
                for s in range(4):
                    t_glob = h * 4 + s
                    ps = bigps.tile([P, 512], mybir.dt.float32, name="ps_big")
                    for q in range(2):
                        nc.tensor.matmul(
                            out=ps[:], lhsT=h1p[q][:, :, s * P:(s + 1) * P],
                            rhs=W7[q][:, :, :], start=(q == 0), stop=(q == 1),
                            perf_mode=DR)
                    tmp = tmpp.tile([P, 512], f16, name="tmp16")
                    nc.vector.tensor_tensor(out=tmp[:], in0=ps[:], in1=n1b2bc[:],
                                            op=ADD)
                    h2 = h2p.tile([P, 512], f16, name="h2t")
                    nc.scalar.activation(h2[:], tmp[:], RELU,
                                         scale=invc[:, t_glob:t_glob + 1])
                    h2_tiles[t_glob] = h2

            def emit_membs(w):
                tlo, thi = win_tiles[w]
                tl = list(range(tlo, thi + 1))
                membs = []
                for t in tl:
                    memb = membp.tile([P, WN], f16, name="memb")
                    nc.vector.tensor_scalar(
                        out=memb[:], in0=iota[:], scalar1=float(WN * w),
                        scalar2=relw[:, t:t + 1], op0=ADD, op1=ISEQ)
                    membs.append(memb)
                return tl, membs

            def emit_window(w, pre=None):
                tl, membs = pre if pre is not None else emit_membs(w)
                seg = segps.tile([P, 512], mybir.dt.float32, name="segps_t", tag="segring")
                for si, t in enumerate(tl):
                    nc.tensor.matmul(out=seg[:], lhsT=membs[si][:],
                                     rhs=h2_tiles[t][:], start=(si == 0),
                                     stop=(si == len(tl) - 1))
                aggT = aggp.tile([P, 512], f16, name="aggT")
                nc.vector.tensor_scalar(out=aggT[:], in0=seg[:], scalar1=1.0,
                                        scalar2=None, op0=MULT)
                pst = segps.tile([P, 4, P], f16, name="ps_tr", tag="segring")
                for fc in range(4):
                    nc.tensor.transpose(pst[:, fc, :],
                                        aggT[:, fc * P:(fc + 1) * P], ident[:])
                aggF = aggp.tile([P, 4, P], f16, name="aggF")
                nc.scalar.activation(aggF[:, :, :], pst[:, :, :], COPY)
                agg = [aggF[:, k, :] for k in range(4)]
                # ---- node MLP2 on this 128-node window ----
                n0 = w * WN
                xw = xn[:, n0:n0 + WN]
                z1 = []
                for m in range(4):
                    ps = bigps.tile([P, 512], mybir.dt.float32, name="ps_big")
                    pss = ps[:, :WN]
                    nc.tensor.matmul(out=pss, lhsT=W8c[:, m * P:(m + 1) * P],
                                     rhs=xw, start=True, stop=False)
                    for k in range(4):
                        nc.tensor.matmul(out=pss, lhsT=W8a[k][:, m * P:(m + 1) * P],
                                         rhs=agg[k], start=False, stop=(k == 3))
                    zt = zp.tile([P, WN], f16, name="z1t")
                    nc.scalar.activation(zt[:], pss, RELU)
                    z1.append(zt)
                ps = bigps.tile([P, 512], mybir.dt.float32, name="ps_big")
                pss = ps[:1, :WN]
                for k in range(4):
                    nc.tensor.matmul(out=pss, lhsT=W9[k][:], rhs=z1[k][:],
                                     start=(k == 0), stop=(k == 3))
                zo = zp.tile([1, WN], f32, name="zot")
                nc.scalar.activation(zo[:], pss, IDENT, bias=B["n2b2r"][:])
                nc.sync.dma_start(out=d_z[:, n0:n0 + WN], in_=zo[:])

            # window w ready once tile win_tiles[w][1] is produced (half t//4)
            ready = {}
            for w in range(NWIN):
                h_ready = min(NH - 1, win_tiles[w][1] // 4)
                ready.setdefault(h_ready, []).append(w)
            pending = []

            staged = []

            def stage_window():
                if pending and not staged:
                    w = pending.pop(0)
                    staged.append((w, emit_membs(w)))

            def make_filler():
                if staged:
                    w, pre = staged.pop(0)
                    emit_window(w, pre)

            einh_of = {}
            LOOK = 4
            for h0 in range(min(LOOK, NH)):
                einh_of[h0] = emit_L1(h0)
            for h in range(NH):
                if h + LOOK < NH:
                    einh_of[h + LOOK] = emit_L1(h + LOOK)
                stage_window()
                emit_rest(h, einh_of.pop(h), make_filler)
                pending.extend(ready.get(h, []))
            while pending:
                emit_window(pending.pop(0))

    nc.compile()
    return nc, names


def kernel(x, edge_attr, u, edge_index, batch, Wsel, bsel,
           eW1, eb1, eW2, eb2, eW3, eb3, eW4, eb4, eW5, eb5,
           n1W1, n1b1, n1W2, n1b2, n2W1, n2b1, n2W2, n2b2):
    f32 = np.float32
    f16 = np.float16
    x = np.asarray(x, f32)
    edge_attr = np.asarray(edge_attr, f32)
    u = np.asarray(u, f32)
    edge_index = np.asarray(edge_index)
    batch = np.asarray(batch)
    ws = {k: np.asarray(v, f32) for k, v in dict(
        Wsel=Wsel, bsel=bsel, eW1=eW1, eb1=eb1, eW2=eW2, eb2=eb2, eW3=eW3,
        eb3=eb3, eW4=eW4, eb4=eb4, eW5=eW5, eb5=eb5, n1W1=n1W1, n1b1=n1b1,
        n1W2=n1W2, n1b2=n1b2, n2W1=n2W1, n2b1=n2b1, n2W2=n2W2, n2b2=n2b2).items()}

    # ---------------- host math (index plumbing + tiny matmuls) -----------
    u2 = (u @ ws["Wsel"] + ws["bsel"]).astype(f32)          # [64, 256]
    row = np.asarray(edge_index[0], np.int64)
    col = np.asarray(edge_index[1], np.int64)
    order = np.argsort(row, kind="stable")
    row_s, col_s = row[order], col[order]
    g_s = np.asarray(batch[row_s], np.int64)
    ea_s = edge_attr[order, 0]
    cnt = np.bincount(row, minlength=N_NODES).astype(f32)
    invc_node = (1.0 / np.maximum(cnt, 1.0)).astype(f32)

    bounds = np.searchsorted(row_s, np.arange(0, N_NODES + 1, NPN))
    e_cnt = np.diff(bounds)
    EPAD = int(-(-int(e_cnt.max()) // 1024) * 1024)
    T = EPAD // P

    # per-window tile ranges (shared across cores) for the static program
    tlo = np.full(NWIN, T - 1, np.int64)
    thi = np.zeros(NWIN, np.int64)
    core_dat = []
    for c in range(NC):
        lo, hi = bounds[c], bounds[c + 1]
        n = hi - lo
        rel = np.full(EPAD, 30000.0, f32)
        rel[:n] = (row_s[lo:hi] - NPN * c).astype(f32)
        w_of_edge = np.floor_divide(rel[:n].astype(np.int64), WN)
        for w in range(NWIN):
            idx = np.nonzero(w_of_edge == w)[0]
            if idx.size:
                tlo[w] = min(tlo[w], idx[0] // P)
                thi[w] = max(thi[w], idx[-1] // P)
        core_dat.append((lo, hi, n, rel))
    win_tiles = [(int(tlo[w]), int(max(tlo[w], thi[w]))) for w in range(NWIN)]
    max_span = max(hw - lw + 1 for lw, hw in win_tiles)
    h2_bufs = max_span + 20
    memb_bufs = max_span + 6

    # ---------------- folded weights ---------------------------------------
    # edge L1: combined K=128 tile.
    # ein rows: 0:9 x[col], 9:18 x[row], 18 ea, 19 const1, 20:84 onehot(g)
    Gb1 = u2 @ ws["eW1"][19:275]                             # [64, 1024]
    W1c = np.zeros((P, E_HID), f32)
    W1c[0:9] = ws["eW1"][9:18]
    W1c[9:18] = ws["eW1"][0:9]
    W1c[18] = ws["eW1"][18]
    W1c[19] = ws["eb1"]
    W1c[20:84] = Gb1
    # node MLP1 L1: fold e5 layer: W_comb = eW5 @ n1W1_e, bias fold
    Wcomb = ws["eW5"] @ ws["n1W1"][9:521]                    # [1024, 512]
    n1b1f = ws["n1b1"] + ws["eb5"] @ ws["n1W1"][9:521]       # [512]
    W6c = np.zeros((P, N_HID), f32)
    W6c[0:9] = ws["n1W1"][0:9] * 64.0
    W6c[19] = n1b1f * 64.0
    # node MLP2: xn rows: 0:9 x, 9 const1, 10:74 onehot(batch)
    Gb2 = u2 @ ws["n2W1"][521:777]                           # [64, 512]
    W8c = np.zeros((P, N_HID), f32)
    W8c[0:9] = ws["n2W1"][0:9]
    W8c[9] = ws["n2b1"]
    W8c[10:74] = Gb2

    def br(b, nm):   # bias [nm*128] -> [128, nm]
        return np.ascontiguousarray(b.reshape(nm, P).T).astype(f32)

    import ml_dtypes
    fp8 = ml_dtypes.float8_e4m3

    def packdr(W):   # [K, M] -> [128, (K//128)*M] fp8, x64, (q,j,m) free order
        K, M = W.shape
        Wp = (W * 64.0).reshape(K // 256, 2, P, M)
        return np.ascontiguousarray(
            np.transpose(Wp, (2, 0, 1, 3)).reshape(P, (K // P) * M)).astype(fp8)

    shared = dict(
        W1c=W1c.astype(f16), eW2=packdr(ws["eW2"]),
        eW3=packdr(ws["eW3"]), eW4=packdr(ws["eW4"]),
        Wcomb=packdr(Wcomb), W6c=W6c.astype(f16),
        n1W2=packdr(ws["n1W2"]),
        W8c=W8c.astype(f16), W8a=ws["n2W1"][9:521].astype(f16),
        W9=ws["n2W2"].astype(f16),
        eb2r=br(ws["eb2"], 8), eb3r=br(ws["eb3"], 8), eb4r=br(ws["eb4"], 8),
        n2b2r=ws["n2b2"].reshape(1, 1).astype(f32),
        iota=np.tile(np.arange(WN, dtype=f16), (P, 1)),
        ident=np.eye(P, dtype=f16),
        n1b2bc=np.tile(ws["n1b2"].astype(f32) * 64.0, (P, 1)),
    )

    in_maps = []
    for c in range(NC):
        lo, hi, n, rel = core_dat[c]
        ein = np.zeros((P, EPAD), f16)
        ein[0:9, :n] = x[col_s[lo:hi]].T
        ein[9:18, :n] = x[row_s[lo:hi]].T
        ein[18, :n] = ea_s[lo:hi]
        ein[19, :n] = 1.0
        oh = np.zeros((N_GRAPHS, n), f16)
        oh[g_s[lo:hi], np.arange(n)] = 1.0
        ein[20:84, :n] = oh
        relw = _pack_cols(rel, T).astype(f32)
        invc_e = np.ones(EPAD, f32)
        invc_e[:n] = invc_node[row_s[lo:hi]]
        invc_e *= 1.0 / 64.0
        xnr = np.zeros((P, NPAD), f16)
        xnr[0:9, :NPN] = x[NPN * c:NPN * (c + 1)].T
        xnr[9, :NPN] = 1.0
        ohn = np.zeros((N_GRAPHS, NPN), f16)
        ohn[np.asarray(batch[NPN * c:NPN * (c + 1)], np.int64),
            np.arange(NPN)] = 1.0
        xnr[10:74, :NPN] = ohn
        im = dict(shared)
        im.update(ein=ein, relw=relw, invc=_pack_cols(invc_e, T), xn=xnr)
        in_maps.append(im)

    # ---------------- build + run ------------------------------------------
    key = (EPAD, tuple(win_tiles))
    if key not in _CACHE:
        _CACHE[key] = _build_module(EPAD, win_tiles, h2_bufs, memb_bufs)
    nc, names = _CACHE[key]

    from concourse import bass_utils
    trace = bool(int(os.environ.get("KERNEL_TRACE", "0")))
    if trace:
        try:
            import types
            import antenv
            if not hasattr(antenv, "axon_hooks"):
                mod = types.ModuleType("antenv.axon_hooks")
                mod._hook = None
                mod.set_axon_ntff_profile_hook = lambda h: setattr(mod, "_hook", h)
                mod.get_axon_ntff_profile_hook = lambda: mod._hook
                sys.modules["antenv.axon_hooks"] = mod
                antenv.axon_hooks = mod
                from trn_agent_boot.trn_boot import _ntff_profile_via_ctypes
                mod._hook = _ntff_profile_via_ctypes("/opt/axon/libaxon_pjrt.so")
        except Exception as e:  # profiling is best-effort
            print("ntff hook shim failed:", e)
            trace = False
    real_maps = [{names[k]: v for k, v in im.items()} for im in in_maps]
    res = bass_utils.run_bass_kernel_spmd(
        nc, real_maps, core_ids=list(range(NC)), trace=trace)
    if trace and res.exec_time_ns is not None:
        print(f"HW exec time: {res.exec_time_ns} ns")
        if res.instructions_and_trace:
            print("trace:", res.instructions_and_trace[1])

    out = np.empty(N_NODES, f32)
    for c in range(NC):
        out[NPN * c:NPN * (c + 1)] = res.results[c][names["zout"]][0, :NPN]
    return out


# revision 24
# speedup vs baseline: 1.0053x; 1.0053x over previous
"""OGRENet GNN message-passing kernel for 8 Trainium2 NeuronCores.

Strategy (v2)
-------------
Host (numpy, cheap index plumbing only):
  * u2 = u @ Wsel + bsel (64x256). Graph-level folds: the u2 contribution to
    edge-MLP L1 is (u2 @ eW1_u)[g] over only 64 graphs -> a 64x1024 bias
    table selected on-device by a one-hot(graph) block inside the L1 K-tile.
    Same trick for node-MLP2's u2 part (64x512 table). Biases ride in a
    const-1 row of the same K-tile, so L1 needs ONE K=128 fp16 matmul per
    M-tile (was 3).
  * eW5 feeds only node-MLP1 -> fold W_comb = eW5 @ n1W1[9:521] on host and
    skip the e5 layer entirely (bias folded into n1b1').
  * sort edges by destination (`row`), split at node boundaries into 8
    contiguous ranges (2500 nodes/core) -> no cross-core reduction.

Device (per core, identical program, different data):
  * edge MLP L2..L4 as fp8 DoubleRow matmuls (weights x64, activations raw
    fp8). ACT does the biased relus into fp8 pairs; L1/n1a relus are split
    ACT/DVE so psum-bank drains keep pace with the PE.
  * node MLP1 L2 flips edge-major; segment mean becomes a matmul with a
    membership mask as the stationary operand (DVE is_equal builds the mask
    with the window shift folded into the op), 128-node windows; agg is
    flipped back feature-major with PE transposes for node-MLP2.
  * emission is software-pipelined with a 4-half L1 lookahead; ready window
    work is interleaved at the L3/L4 boundary and membership masks are
    front-loaded in the DVE queue so the PE instruction stream stays
    gapless (any idle also drops the PE p-state, doubling the cost).
"""

import os
import sys

import numpy as np

sys.path.insert(0, "/opt/trn_rl_repo")

N_NODES = 20000
N_GRAPHS = 64
U_DIM = 256
E_HID = 1024
N_HID = 512
NC = 8
NPN = N_NODES // NC          # nodes per core (2500)
NPAD = 2560                  # padded nodes per core
WN = 128                     # nodes per segment window
NWIN = NPAD // WN            # 20
P = 128

_CACHE = {}


def _pack_cols(v, T):
    """[T*128] -> [128, T] with col t = v[t*128:(t+1)*128]."""
    return np.ascontiguousarray(v.reshape(T, P).T)


def _build_module(EPAD, win_tiles, h2_bufs, memb_bufs):
    """Build the per-core Bass program. win_tiles[w] = (tlo, thi) inclusive."""
    from concourse import bacc, mybir, tile

    T = EPAD // P            # 128-edge tiles
    NB = EPAD // 1024        # 1024-edge dma blocks
    NH = EPAD // 512         # 512-edge halves
    f16 = mybir.dt.float16
    f32 = mybir.dt.float32
    f8 = mybir.dt.float8e4
    RELU = mybir.ActivationFunctionType.Relu
    COPY = mybir.ActivationFunctionType.Copy
    IDENT = mybir.ActivationFunctionType.Identity
    DR = mybir.MatmulPerfMode.DoubleRow
    ADD = mybir.AluOpType.add
    MAX = mybir.AluOpType.max
    MULT = mybir.AluOpType.mult
    ISEQ = mybir.AluOpType.is_equal
    INV64 = 1.0 / 64.0

    nc = bacc.Bacc(None, target_bir_lowering=False, debug=False)

    with tile.TileContext(nc) as tc:
        with (
            tc.tile_pool(name="dram", bufs=1, space="DRAM") as dram,
            tc.tile_pool(name="wres", bufs=1) as wres,
            tc.tile_pool(name="einp", bufs=4) as einp,
            tc.tile_pool(name="actp", bufs=16) as actp,
            tc.tile_pool(name="a1pool", bufs=20) as a1pool,
            tc.tile_pool(name="h2p", bufs=h2_bufs) as h2p,
            tc.tile_pool(name="membp", bufs=memb_bufs) as membp,
            tc.tile_pool(name="tmpp", bufs=3) as tmpp,
            tc.tile_pool(name="zp", bufs=6) as zp,
            tc.tile_pool(name="aggp", bufs=8) as aggp,
            tc.tile_pool(name="bigps", bufs=6, space="PSUM") as bigps,
            tc.tile_pool(name="segps", bufs=2, space="PSUM") as segps,
        ):
            # ---- DRAM I/O -------------------------------------------------
            d_ein = dram.tile([P, EPAD], f16, kind="ExternalInput", name="ein")
            d_relw = dram.tile([P, T], f32, kind="ExternalInput", name="relw")
            d_invc = dram.tile([P, T], f32, kind="ExternalInput", name="invc")
            d_xn = dram.tile([P, NPAD], f16, kind="ExternalInput", name="xn")
            d_iota = dram.tile([P, WN], f16, kind="ExternalInput", name="iota")
            d_ident = dram.tile([P, P], f16, kind="ExternalInput", name="ident")
            d_n1b2bc = dram.tile([P, 512], f32, kind="ExternalInput", name="n1b2bc")

            wspec16 = dict(W1c=[P, E_HID], W6c=[P, N_HID], W8c=[P, N_HID],
                           W8a=[N_HID, N_HID], W9=[N_HID, 1])
            d_w = {k: dram.tile(s, f16, kind="ExternalInput", name=k)
                   for k, s in wspec16.items()}
            for k in ("eW2", "eW3", "eW4"):
                d_w[k] = dram.tile([P, 8192], f8, kind="ExternalInput", name=k)
            d_w["Wcomb"] = dram.tile([P, 4096], f8, kind="ExternalInput", name="Wcomb")
            d_w["n1W2"] = dram.tile([P, 2048], f8, kind="ExternalInput", name="n1W2")
            bspec = dict(eb2r=[P, 8], eb3r=[P, 8], eb4r=[P, 8], n2b2r=[1, 1])
            d_b = {k: dram.tile(s, f32, kind="ExternalInput", name=k)
                   for k, s in bspec.items()}
            d_z = dram.tile([1, NPAD], f32, kind="ExternalOutput", name="zout")

            names = dict(ein=d_ein.name, relw=d_relw.name, invc=d_invc.name,
                         xn=d_xn.name, iota=d_iota.name, ident=d_ident.name,
                         n1b2bc=d_n1b2bc.name, zout=d_z.name)
            names.update({k: v.name for k, v in d_w.items()})
            names.update({k: v.name for k, v in d_b.items()})

            # ---- resident loads ------------------------------------------
            def load_w(name, nk, width):
                ts = []
                for k in range(nk):
                    t = wres.tile([P, width], f16, name=f"w_{name}_{k}")
                    nc.sync.dma_start(out=t[:], in_=d_w[name][k * P:(k + 1) * P, :])
                    ts.append(t)
                return ts

            def load_wdr(name, npair, width):
                ts = []
                for q in range(npair):
                    t = wres.tile([P, 2, width], f8, name=f"w_{name}_{q}")
                    nc.sync.dma_start(
                        out=t[:, :, :],
                        in_=d_w[name][:, q * 2 * width:(q + 1) * 2 * width])
                    ts.append(t)
                return ts

            # prefetch the first two edge-input blocks ahead of the bulkier
            # weight loads so L1(0) isn't DMA-bound at startup
            ein_blocks = {}
            for b0 in range(min(4, NB)):
                t = einp.tile([P, 1024], f16, name="ein_t")
                for hh in range(2):
                    nc.sync.dma_start(
                        out=t[:, hh * 512:(hh + 1) * 512],
                        in_=d_ein[:, b0 * 1024 + hh * 512:
                                  b0 * 1024 + (hh + 1) * 512])
                ein_blocks[b0] = t

            W1c = load_w("W1c", 1, E_HID)[0]
            W2 = load_wdr("eW2", 4, E_HID)
            W3 = load_wdr("eW3", 4, E_HID)
            W4 = load_wdr("eW4", 4, E_HID)
            Wcb = load_wdr("Wcomb", 4, N_HID)
            W6c = load_w("W6c", 1, N_HID)[0]
            W7 = load_wdr("n1W2", 2, N_HID)
            W8c = load_w("W8c", 1, N_HID)[0]
            W8a = load_w("W8a", 4, N_HID)
            W9 = load_w("W9", 4, 1)

            B = {}
            for k, s in bspec.items():
                t = wres.tile(s, f32, name=f"b_{k}")
                nc.sync.dma_start(out=t[:], in_=d_b[k][:])
                B[k] = t
            relw = wres.tile([P, T], f32, name="relw_sb")
            nc.sync.dma_start(out=relw[:], in_=d_relw[:])
            invc = wres.tile([P, T], f32, name="invc_sb")
            nc.sync.dma_start(out=invc[:], in_=d_invc[:])
            iota = wres.tile([P, WN], f16, name="iota_sb")
            nc.sync.dma_start(out=iota[:], in_=d_iota[:])
            ident = wres.tile([P, P], f16, name="ident_sb")
            nc.sync.dma_start(out=ident[:], in_=d_ident[:])
            n1b2bc = wres.tile([P, 512], f32, name="n1b2bc_sb")
            nc.sync.dma_start(out=n1b2bc[:], in_=d_n1b2bc[:])
            xn = wres.tile([P, NPAD], f16, name="xn_sb")
            nc.sync.dma_start(out=xn[:], in_=d_xn[:])

            # ---- state ----------------------------------------------------
            a1_pairs = {}      # h -> [4 pair tiles]
            h2_tiles = {}      # global tile idx -> tile

            def emit_L1(h):
                b = h // 2
                if h % 2 == 0 and b not in ein_blocks:
                    t = einp.tile([P, 1024], f16, name="ein_t")
                    nc.sync.dma_start(out=t[:],
                                      in_=d_ein[:, b * 1024:(b + 1) * 1024])
                    ein_blocks[b] = t
                einh = ein_blocks[b][:, (h % 2) * 512:(h % 2) * 512 + 512]
                pairs = []
                pt = None
                for m in range(8):
                    ps = bigps.tile([P, 512], mybir.dt.float32, name="ps_big")
                    nc.tensor.matmul(out=ps[:], lhsT=W1c[:, m * P:(m + 1) * P],
                                     rhs=einh, start=True, stop=True)
                    if m % 2 == 0:
                        pt = a1pool.tile([P, 2, 512], f8, name="pairA")
                        pairs.append(pt)
                    if m in (0, 1, 4, 5):
                        nc.scalar.activation(pt[:, m % 2, :], ps[:], RELU)
                    else:
                        nc.vector.tensor_scalar(
                            out=pt[:, m % 2, :], in0=ps[:], scalar1=0.0,
                            scalar2=None, op0=MAX)
                a1_pairs[h] = pairs
                return einh

            def dr_layer(pin, Wp, bias, tag):
                outs = []
                pt = None
                # interleave the first two M-groups so the group-0 q=3 matmul
                # lands after the previous layer's last activation drains
                ps0 = bigps.tile([P, 512], mybir.dt.float32, name="ps_big")
                ps1 = bigps.tile([P, 512], mybir.dt.float32, name="ps_big")
                for q in range(4):
                    for m, ps in ((0, ps0), (1, ps1)):
                        nc.tensor.matmul(
                            out=ps[:], lhsT=Wp[q][:, :, m * P:(m + 1) * P],
                            rhs=pin[q][:, :, :], start=(q == 0),
                            stop=(q == 3), perf_mode=DR)
                for m in range(8):
                    if m == 0:
                        ps = ps0
                    elif m == 1:
                        ps = ps1
                    else:
                        ps = bigps.tile([P, 512], mybir.dt.float32, name="ps_big")
                        for q in range(4):
                            nc.tensor.matmul(
                                out=ps[:], lhsT=Wp[q][:, :, m * P:(m + 1) * P],
                                rhs=pin[q][:, :, :], start=(q == 0),
                                stop=(q == 3), perf_mode=DR)
                    if m % 2 == 0:
                        pt = actp.tile([P, 2, 512], f8, name=tag)
                        outs.append(pt)
                    nc.scalar.activation(pt[:, m % 2, :], ps[:], RELU,
                                         bias=bias[:, m:m + 1], scale=INV64)
                return outs

            def emit_rest(h, einh, emit_filler):
                a2 = dr_layer(a1_pairs.pop(h), W2, B["eb2r"], "pairB")
                a3 = dr_layer(a2, W3, B["eb3r"], "pairC")
                emit_filler()
                a4 = dr_layer(a3, W4, B["eb4r"], "pairB")
                # node MLP1 L1: fp16 x/bias part + fp8 DR W_comb on a4
                h1p = []
                pt = None
                for m in range(4):
                    ps = bigps.tile([P, 512], mybir.dt.float32, name="ps_big")
                    nc.tensor.matmul(out=ps[:], lhsT=W6c[:, m * P:(m + 1) * P],
                                     rhs=einh, start=True, stop=False)
                    for q in range(4):
                        nc.tensor.matmul(
                            out=ps[:], lhsT=Wcb[q][:, :, m * P:(m + 1) * P],
                            rhs=a4[q][:, :, :], start=False, stop=(q == 3),
                            perf_mode=DR)
                    if m % 2 == 0:
                        pt = actp.tile([P, 2, 512], f8, name="pairD")
                        h1p.append(pt)
                    nc.vector.tensor_scalar(
                        out=pt[:, m % 2, :], in0=ps[:], scalar1=INV64,
                        scalar2=0.0, op0=MULT, op1=MAX)
                # node MLP1 L2 -> edge-major h2 tiles
                for s in range(4):
                    t_glob = h * 4 + s
                    ps = bigps.tile([P, 512], mybir.dt.float32, name="ps_big")
                    for q in range(2):
                        nc.tensor.matmul(
                            out=ps[:], lhsT=h1p[q][:, :, s * P:(s + 1) * P],
                            rhs=W7[q][:, :, :], start=(q == 0), stop=(q == 1),
                            perf_mode=DR)
                    tmp = tmpp.tile([P, 512], f16, name="tmp16")
                    nc.vector.tensor_tensor(out=tmp[:], in0=ps[:], in1=n1b2bc[:],
                                            op=ADD)
                    h2 = h2p.tile([P, 512], f16, name="h2t")
                    nc.scalar.activation(h2[:], tmp[:], RELU,
                                         scale=invc[:, t_glob:t_glob + 1])
                    h2_tiles[t_glob] = h2

            def emit_membs(w):
                tlo, thi = win_tiles[w]
                tl = list(range(tlo, thi + 1))
                membs = []
                for t in tl:
                    memb = membp.tile([P, WN], f16, name="memb")
                    nc.vector.tensor_scalar(
                        out=memb[:], in0=iota[:], scalar1=float(WN * w),
                        scalar2=relw[:, t:t + 1], op0=ADD, op1=ISEQ)
                    membs.append(memb)
                return tl, membs

            def emit_window(w, pre=None):
                tl, membs = pre if pre is not None else emit_membs(w)
                seg = segps.tile([P, 512], mybir.dt.float32, name="segps_t", tag="segring")
                for si, t in enumerate(tl):
                    nc.tensor.matmul(out=seg[:], lhsT=membs[si][:],
                                     rhs=h2_tiles[t][:], start=(si == 0),
                                     stop=(si == len(tl) - 1))
                aggT = aggp.tile([P, 512], f16, name="aggT")
                nc.vector.tensor_scalar(out=aggT[:], in0=seg[:], scalar1=1.0,
                                        scalar2=None, op0=MULT)
                pst = segps.tile([P, 4, P], f16, name="ps_tr", tag="segring")
                for fc in range(4):
                    nc.tensor.transpose(pst[:, fc, :],
                                        aggT[:, fc * P:(fc + 1) * P], ident[:])
                aggF = aggp.tile([P, 4, P], f16, name="aggF")
                nc.scalar.activation(aggF[:, :, :], pst[:, :, :], COPY)
                agg = [aggF[:, k, :] for k in range(4)]
                # ---- node MLP2 on this 128-node window ----
                n0 = w * WN
                xw = xn[:, n0:n0 + WN]
                z1 = []
                for m in range(4):
                    ps = bigps.tile([P, 512], mybir.dt.float32, name="ps_big")
                    pss = ps[:, :WN]
                    nc.tensor.matmul(out=pss, lhsT=W8c[:, m * P:(m + 1) * P],
                                     rhs=xw, start=True, stop=False)
                    for k in range(4):
                        nc.tensor.matmul(out=pss, lhsT=W8a[k][:, m * P:(m + 1) * P],
                                         rhs=agg[k], start=False, stop=(k == 3))
                    zt = zp.tile([P, WN], f16, name="z1t")
                    nc.scalar.activation(zt[:], pss, RELU)
                    z1.append(zt)
                ps = bigps.tile([P, 512], mybir.dt.float32, name="ps_big")
                pss = ps[:1, :WN]
                for k in range(4):
                    nc.tensor.matmul(out=pss, lhsT=W9[k][:], rhs=z1[k][:],
                                     start=(k == 0), stop=(k == 3))
                zo = zp.tile([1, WN], f32, name="zot")
                nc.scalar.activation(zo[:], pss, IDENT, bias=B["n2b2r"][:])
                nc.sync.dma_start(out=d_z[:, n0:n0 + WN], in_=zo[:])

            # window w ready once tile win_tiles[w][1] is produced (half t//4)
            ready = {}
            for w in range(NWIN):
                h_ready = min(NH - 1, win_tiles[w][1] // 4)
                ready.setdefault(h_ready, []).append(w)
            pending = []

            staged = []

            def stage_window():
                if pending and not staged:
                    w = pending.pop(0)
                    staged.append((w, emit_membs(w)))

            def make_filler():
                if staged:
                    w, pre = staged.pop(0)
                    emit_window(w, pre)

            einh_of = {}
            LOOK = 4
            for h0 in range(min(LOOK, NH)):
                einh_of[h0] = emit_L1(h0)
            for h in range(NH):
                if h + LOOK < NH:
                    einh_of[h + LOOK] = emit_L1(h + LOOK)
                stage_window()
                emit_rest(h, einh_of.pop(h), make_filler)
                pending.extend(ready.get(h, []))
            while pending:
                emit_window(pending.pop(0))

    nc.compile()
    return nc, names


def kernel(x, edge_attr, u, edge_index, batch, Wsel, bsel,
           eW1, eb1, eW2, eb2, eW3, eb3, eW4, eb4, eW5, eb5,
           n1W1, n1b1, n1W2, n1b2, n2W1, n2b1, n2W2, n2b2):
    f32 = np.float32
    f16 = np.float16
    x = np.asarray(x, f32)
    edge_attr = np.asarray(edge_attr, f32)
    u = np.asarray(u, f32)
    edge_index = np.asarray(edge_index)
    batch = np.asarray(batch)
    ws = {k: np.asarray(v, f32) for k, v in dict(
        Wsel=Wsel, bsel=bsel, eW1=eW1, eb1=eb1, eW2=eW2, eb2=eb2, eW3=eW3,
        eb3=eb3, eW4=eW4, eb4=eb4, eW5=eW5, eb5=eb5, n1W1=n1W1, n1b1=n1b1,
        n1W2=n1W2, n1b2=n1b2, n2W1=n2W1, n2b1=n2b1, n2W2=n2W2, n2b2=n2b2).items()}

    # ---------------- host math (index plumbing + tiny matmuls) -----------
    u2 = (u @ ws["Wsel"] + ws["bsel"]).astype(f32)          # [64, 256]
    row = np.asarray(edge_index[0], np.int64)
    col = np.asarray(edge_index[1], np.int64)
    order = np.argsort(row, kind="stable")
    row_s, col_s = row[order], col[order]
    g_s = np.asarray(batch[row_s], np.int64)
    ea_s = edge_attr[order, 0]
    cnt = np.bincount(row, minlength=N_NODES).astype(f32)
    invc_node = (1.0 / np.maximum(cnt, 1.0)).astype(f32)

    bounds = np.searchsorted(row_s, np.arange(0, N_NODES + 1, NPN))
    e_cnt = np.diff(bounds)
    EPAD = int(-(-int(e_cnt.max()) // 1024) * 1024)
    T = EPAD // P

    # per-window tile ranges (shared across cores) for the static program
    tlo = np.full(NWIN, T - 1, np.int64)
    thi = np.zeros(NWIN, np.int64)
    core_dat = []
    for c in range(NC):
        lo, hi = bounds[c], bounds[c + 1]
        n = hi - lo
        rel = np.full(EPAD, 30000.0, f32)
        rel[:n] = (row_s[lo:hi] - NPN * c).astype(f32)
        w_of_edge = np.floor_divide(rel[:n].astype(np.int64), WN)
        for w in range(NWIN):
            idx = np.nonzero(w_of_edge == w)[0]
            if idx.size:
                tlo[w] = min(tlo[w], idx[0] // P)
                thi[w] = max(thi[w], idx[-1] // P)
        core_dat.append((lo, hi, n, rel))
    win_tiles = [(int(tlo[w]), int(max(tlo[w], thi[w]))) for w in range(NWIN)]
    max_span = max(hw - lw + 1 for lw, hw in win_tiles)
    h2_bufs = max_span + 20
    memb_bufs = max_span + 6

    # ---------------- folded weights ---------------------------------------
    # edge L1: combined K=128 tile.
    # ein rows: 0:9 x[col], 9:18 x[row], 18 ea, 19 const1, 20:84 onehot(g)
    Gb1 = u2 @ ws["eW1"][19:275]                             # [64, 1024]
    W1c = np.zeros((P, E_HID), f32)
    W1c[0:9] = ws["eW1"][9:18]
    W1c[9:18] = ws["eW1"][0:9]
    W1c[18] = ws["eW1"][18]
    W1c[19] = ws["eb1"]
    W1c[20:84] = Gb1
    # node MLP1 L1: fold e5 layer: W_comb = eW5 @ n1W1_e, bias fold
    Wcomb = ws["eW5"] @ ws["n1W1"][9:521]                    # [1024, 512]
    n1b1f = ws["n1b1"] + ws["eb5"] @ ws["n1W1"][9:521]       # [512]
    W6c = np.zeros((P, N_HID), f32)
    W6c[0:9] = ws["n1W1"][0:9] * 64.0
    W6c[19] = n1b1f * 64.0
    # node MLP2: xn rows: 0:9 x, 9 const1, 10:74 onehot(batch)
    Gb2 = u2 @ ws["n2W1"][521:777]                           # [64, 512]
    W8c = np.zeros((P, N_HID), f32)
    W8c[0:9] = ws["n2W1"][0:9]
    W8c[9] = ws["n2b1"]
    W8c[10:74] = Gb2

    def br(b, nm):   # bias [nm*128] -> [128, nm]
        return np.ascontiguousarray(b.reshape(nm, P).T).astype(f32)

    import ml_dtypes
    fp8 = ml_dtypes.float8_e4m3

    def packdr(W):   # [K, M] -> [128, (K//128)*M] fp8, x64, (q,j,m) free order
        K, M = W.shape
        Wp = (W * 64.0).reshape(K // 256, 2, P, M)
        return np.ascontiguousarray(
            np.transpose(Wp, (2, 0, 1, 3)).reshape(P, (K // P) * M)).astype(fp8)

    shared = dict(
        W1c=W1c.astype(f16), eW2=packdr(ws["eW2"]),
        eW3=packdr(ws["eW3"]), eW4=packdr(ws["eW4"]),
        Wcomb=packdr(Wcomb), W6c=W6c.astype(f16),
        n1W2=packdr(ws["n1W2"]),
        W8c=W8c.astype(f16), W8a=ws["n2W1"][9:521].astype(f16),
        W9=ws["n2W2"].astype(f16),
        eb2r=br(ws["eb2"], 8), eb3r=br(ws["eb3"], 8), eb4r=br(ws["eb4"], 8),
        n2b2r=ws["n2b2"].reshape(1, 1).astype(f32),
        iota=np.tile(np.arange(WN, dtype=f16), (P, 1)),
        ident=np.eye(P, dtype=f16),
        n1b2bc=np.tile(ws["n1b2"].astype(f32) * 64.0, (P, 1)),
    )

    in_maps = []
    for c in range(NC):
        lo, hi, n, rel = core_dat[c]
        ein = np.zeros((P, EPAD), f16)
        ein[0:9, :n] = x[col_s[lo:hi]].T
        ein[9:18, :n] = x[row_s[lo:hi]].T
        ein[18, :n] = ea_s[lo:hi]
        ein[19, :n] = 1.0
        oh = np.zeros((N_GRAPHS, n), f16)
        oh[g_s[lo:hi], np.arange(n)] = 1.0
        ein[20:84, :n] = oh
        relw = _pack_cols(rel, T).astype(f32)
        invc_e = np.ones(EPAD, f32)
        invc_e[:n] = invc_node[row_s[lo:hi]]
        invc_e *= 1.0 / 64.0
        xnr = np.zeros((P, NPAD), f16)
        xnr[0:9, :NPN] = x[NPN * c:NPN * (c + 1)].T
        xnr[9, :NPN] = 1.0
        ohn = np.zeros((N_GRAPHS, NPN), f16)
        ohn[np.asarray(batch[NPN * c:NPN * (c + 1)], np.int64),
            np.arange(NPN)] = 1.0
        xnr[10:74, :NPN] = ohn
        im = dict(shared)
        im.update(ein=ein, relw=relw, invc=_pack_cols(invc_e, T), xn=xnr)
        in_maps.append(im)

    # ---------------- build + run ------------------------------------------
    key = (EPAD, tuple(win_tiles))
    if key not in _CACHE:
        _CACHE[key] = _build_module(EPAD, win_tiles, h2_bufs, memb_bufs)
    nc, names = _CACHE[key]

    from concourse import bass_utils
    trace = bool(int(os.environ.get("KERNEL_TRACE", "0")))
    if trace:
        try:
            import types
            import antenv
            if not hasattr(antenv, "axon_hooks"):
                mod = types.ModuleType("antenv.axon_hooks")
                mod._hook = None
                mod.set_axon_ntff_profile_hook = lambda h: setattr(mod, "_hook", h)
                mod.get_axon_ntff_profile_hook = lambda: mod._hook
                sys.modules["antenv.axon_hooks"] = mod
                antenv.axon_hooks = mod
                from trn_agent_boot.trn_boot import _ntff_profile_via_ctypes
                mod._hook = _ntff_profile_via_ctypes("/opt/axon/libaxon_pjrt.so")
        except Exception as e:  # profiling is best-effort
            print("ntff hook shim failed:", e)
            trace = False
    real_maps = [{names[k]: v for k, v in im.items()} for im in in_maps]
    res = bass_utils.run_bass_kernel_spmd(
        nc, real_maps, core_ids=list(range(NC)), trace=trace)
    if trace and res.exec_time_ns is not None:
        print(f"HW exec time: {res.exec_time_ns} ns")
        if res.instructions_and_trace:
            print("trace:", res.instructions_and_trace[1])

    out = np.empty(N_NODES, f32)
    for c in range(NC):
        out[NPN * c:NPN * (c + 1)] = res.results[c][names["zout"]][0, :NPN]
    return out
